# revision 43
# baseline (speedup 1.0000x reference)
"""Trainium2 Bass kernel for GQA attention (B=4, S=2048, H=576, 9 heads / 3 KV groups, RoPE).

Sharding: 8 cores = (batch b, seq-half) pairs. Each core computes the full
attention output for 1024 query rows of one batch element (keys/values over
the full 2048 positions of that batch element are recomputed locally; no
collectives needed).

Layout strategy: everything stays "transposed" (features on partitions, seq on
free dim):
  QT = wq @ hsT, KT = wk @ hsT (RoPE applied in T space on DVE)
  V natural [s, hv] via lhsT = hsT chunks; va layout per group = [ones | V64]
  ST[k, q] = KT.T-stationary @ QT (two heads row-tiled, concurrent in PE)
  exp: split between ACT (exact) and DVE (Schraudolph fp16 bit-hack) so both
  engines work in parallel; attnT fp16 in SBUF
  avT[1+hd, q] = [ones | V].T @ attnT  (row 0 = softmax denominator)
  final^T = woT.T-stationary @ (avT[1:65] / avT[0])
Matmul inputs fp16 (fp32 PSUM accumulation), output fp32.
"""

import sys

if "/opt/trn_rl_repo" not in sys.path:
    sys.path.insert(0, "/opt/trn_rl_repo")

import numpy as np

import concourse.bass as bass
import concourse.mybir as mybir
import concourse.tile as tile
from concourse import bacc
from concourse.bass_utils import run_bass_kernel_spmd

F16 = mybir.dt.float16
F32 = mybir.dt.float32
I16 = mybir.dt.int16

B = 4
S = 2048
SQ = 1024  # query rows per core
H = 576
HP = 640  # hidden padded to 5*128
NH = 9
HD = 64
KV = 192
G = 3
ROPE_THETA = 10000.0
SCALE = 1.0 / 8.0  # 1/sqrt(HD)

NDC = HP // 128  # 5 contraction chunks
NEC = 5  # output feature chunks of QT (4*128 + 64)
NKC = S // 128  # 16 key chunks

# --- exp split: which key chunks use the DVE Schraudolph approx-exp ---------
# exp(x) ~ bitcast_f16(int16(round(x*SCHR_A + SCHR_B)));  x = raw score, the
# 1/8 softmax scale is folded into SCHR_A. ~2-3%% relative error, zero-mean-ish
# component cancels in the softmax ratio; validated end-to-end vs tolerance.
# SCHR_A is folded into the Q-side rope tables on the host, so scores arrive
# in PSUM already scaled: the DVE op is a single scalar add (+SCHR_B) and the
# ACT path just uses scale=ln2/1024 instead of 1/8.
DVE_KC = (5, 8, 11, 14)
SCHR_A = (1024.0 / float(np.log(2.0))) * SCALE
SCHR_B = 15360.0 - 44.0
ACT_SCALE = float(np.log(2.0)) / 1024.0


def _rope_tables():
    """fp32 master cos/sin tables [128, S] with dest-indexed sin signs."""
    inv_freq = 1.0 / (ROPE_THETA ** (np.arange(0, HD, 2, dtype=np.float32) / HD))
    t = np.arange(S, dtype=np.float32)
    freqs = np.einsum("i,j->ij", inv_freq, t)  # [32, S]
    cos32 = np.cos(freqs)
    sin32 = np.sin(freqs)
    cos4 = np.tile(cos32, (4, 1))  # [128, S]
    # sin indexed by DEST rows: out[j<32] = q[j]*cos - q[j+32]*sin[j];
    # out[j>=32] = q[j]*cos + q[j-32]*sin. The shifted tile sh[j] holds the
    # cross row, so sign pattern per 64-block is [-sin32; +sin32].
    sin2 = np.concatenate([-sin32, sin32, -sin32, sin32], axis=0)  # [128, S]
    return cos4, sin2


def _build_bass():
    nc = bacc.Bacc("TRN2", target_bir_lowering=False)

    hsT = nc.declare_dram_parameter("hsT", [HP, S], F16, isOutput=False)
    wqT = nc.declare_dram_parameter("wqT", [HP, H], F16, isOutput=False)
    wkT = nc.declare_dram_parameter("wkT", [HP, KV], F16, isOutput=False)
    wvT = nc.declare_dram_parameter("wvT", [HP, KV], F16, isOutput=False)
    woT = nc.declare_dram_parameter("woT", [H, H], F16, isOutput=False)
    cosq = nc.declare_dram_parameter("cosq", [128, SQ], F16, isOutput=False)
    sinq = nc.declare_dram_parameter("sinq", [128, SQ], F16, isOutput=False)
    cosk = nc.declare_dram_parameter("cosk", [128, S], F16, isOutput=False)
    sink = nc.declare_dram_parameter("sink", [128, S], F16, isOutput=False)
    out = nc.declare_dram_parameter("o", [H, SQ], F32, isOutput=True)

    with tile.TileContext(nc) as tc:
        kernel_body(nc, tc, hsT, wqT, wkT, wvT, woT, cosq, sinq, cosk, sink, out)

    nc.compile()
    return nc


def kernel_body(nc, tc, hsT, wqT, wkT, wvT, woT, cosq, sinq, cosk, sink, out):
    import contextlib

    ctx = contextlib.ExitStack()
    with ctx:
        # ---------------- persistent SBUF pools ----------------
        wpool = ctx.enter_context(tc.tile_pool(name="w", bufs=1))
        qtp = ctx.enter_context(tc.tile_pool(name="qt", bufs=1))
        ktp = ctx.enter_context(tc.tile_pool(name="kt", bufs=1))
        vap = ctx.enter_context(tc.tile_pool(name="va", bufs=1))
        otp = ctx.enter_context(tc.tile_pool(name="ot", bufs=1))
        ropep = ctx.enter_context(tc.tile_pool(name="rope", bufs=2))
        attnp = ctx.enter_context(tc.tile_pool(name="attn", bufs=4))
        miscp = ctx.enter_context(tc.tile_pool(name="misc", bufs=4))

        # ---------------- load inputs to SBUF ----------------
        hs_sb = []
        wq_sb = []
        wk_sb = []
        wv_sb = []
        wo_sb = []
        # spread the ~5MB of input loads across per-engine HWDGE queues so
        # they run in parallel instead of serializing on the sync queue
        qeng = [nc.sync, nc.scalar, nc.sync, nc.scalar]
        for dc in range(NDC):
            t = wpool.tile([128, S], F16, tag=f"hs{dc}", name=f"hs{dc}")
            qeng[dc % 4].dma_start(out=t, in_=hsT[dc * 128 : (dc + 1) * 128, :])
            hs_sb.append(t)
            t = wpool.tile([128, H], F16, tag=f"wq{dc}", name=f"wq{dc}")
            qeng[(dc + 1) % 4].dma_start(out=t, in_=wqT[dc * 128 : (dc + 1) * 128, :])
            wq_sb.append(t)
            t = wpool.tile([128, KV], F16, tag=f"wk{dc}", name=f"wk{dc}")
            qeng[(dc + 2) % 4].dma_start(out=t, in_=wkT[dc * 128 : (dc + 1) * 128, :])
            wk_sb.append(t)
            t = wpool.tile([128, KV], F16, tag=f"wv{dc}", name=f"wv{dc}")
            qeng[(dc + 3) % 4].dma_start(out=t, in_=wvT[dc * 128 : (dc + 1) * 128, :])
            wv_sb.append(t)
        for ec in range(NEC):
            m = min(128, H - ec * 128)
            t = wpool.tile([128, H], F16, tag=f"wo{ec}", name=f"wo{ec}")
            qeng[ec % 4].dma_start(out=t[:m, :], in_=woT[ec * 128 : ec * 128 + m, :])
            wo_sb.append(t)
        cosq_sb = wpool.tile([128, SQ], F16, tag="cosq")
        nc.scalar.dma_start(out=cosq_sb, in_=cosq[:, :])
        sinq_sb = wpool.tile([128, SQ], F16, tag="sinq")
        nc.gpsimd.dma_start(out=sinq_sb, in_=sinq[:, :])
        cosk_sb = wpool.tile([128, S], F16, tag="cosk")
        nc.sync.dma_start(out=cosk_sb, in_=cosk[:, :])
        sink_sb = wpool.tile([128, S], F16, tag="sink")
        nc.sync.dma_start(out=sink_sb, in_=sink[:, :])

        # persistent activation tensors
        qt_sb = [qtp.tile([128, SQ], F16, tag=f"qt{c}", name=f"qt{c}") for c in range(NEC)]
        ktd_sb = [ktp.tile([128, S], F16, tag=f"ktd{g}", name=f"ktd{g}") for g in range(G)]
        va_sb = [vap.tile([128, 3 * 65], F16, tag=f"va{kc}", name=f"va{kc}") for kc in range(NKC)]
        ot_sb = [otp.tile([128, SQ], F16, tag=f"ot{c}", name=f"ot{c}") for c in range(NEC)]

        # PSUM pools: st [128,1024] x3 = 6 banks, av [65,512] x2 = 2 banks
        psp = ctx.enter_context(tc.tile_pool(name="ps", bufs=3, space="PSUM"))
        avp = ctx.enter_context(tc.tile_pool(name="avp", bufs=2, space="PSUM"))

        def cast_rope(ps_ap, nrows, width, cos_ap, sin_ap, dst_writes, nm):
            """cast psum->sbuf fp16, then rope via DMA half-swap + 3 DVE ops.

            dst_writes: list of (dst_ap [64 or 128 rows, width], src_row)."""
            raw = ropep.tile([128, width], F16, tag="rraw", name=f"rr{nm}")
            nc.vector.tensor_copy(raw[:nrows], ps_ap)
            sh = ropep.tile([128, width], F16, tag="rsh", name=f"rs{nm}")
            for b0 in range(0, nrows, 64):
                nc.gpsimd.dma_start(out=sh[b0 : b0 + 32], in_=raw[b0 + 32 : b0 + 64])
                nc.gpsimd.dma_start(out=sh[b0 + 32 : b0 + 64], in_=raw[b0 : b0 + 32])
            t1 = ropep.tile([128, width], F16, tag="rt1", name=f"r1{nm}")
            t2 = ropep.tile([128, width], F16, tag="rt2", name=f"r2{nm}")
            nc.vector.tensor_mul(t1[:nrows], raw[:nrows], cos_ap[:nrows])
            nc.vector.tensor_mul(t2[:nrows], sh[:nrows], sin_ap[:nrows])
            for dst, row in dst_writes:
                n = dst.partition_size()
                nc.vector.tensor_add(dst, t1[row : row + n], t2[row : row + n])

        # Queries are always hsT columns [0, SQ): cores covering the second
        # seq half pass hsT (and cos/sin) rolled by -SQ columns.
        QO = 0

        def k_proj(piece, chunks=(0, 1)):
            so = piece * SQ
            for kc_ch, (roff, nh) in enumerate([(0, 2), (128, 1)]):
                if kc_ch not in chunks:
                    continue
                m = nh * 64
                kps = psp.tile([128, SQ], F32, tag="st", name=f"kps{piece}{kc_ch}")
                for dc in range(NDC):
                    for sb2 in range(2):
                        nc.tensor.matmul(
                            kps[:m, sb2 * 512 : (sb2 + 1) * 512],
                            lhsT=wk_sb[dc][:, roff : roff + m],
                            rhs=hs_sb[dc][:, so + sb2 * 512 : so + (sb2 + 1) * 512],
                            start=(dc == 0),
                            stop=(dc == NDC - 1),
                        )
                writes = []
                for h2 in range(nh):
                    g = kc_ch * 2 + h2
                    writes.append((ktd_sb[g][0:64, so : so + SQ], h2 * 64))
                cast_rope(
                    kps[:m],
                    m,
                    SQ,
                    cosk_sb[:, so : so + SQ],
                    sink_sb[:, so : so + SQ],
                    writes,
                    f"k{piece}{kc_ch}",
                )
                # duplicate rows 0-63 -> 64-127 for row-packed score matmuls
                for h2 in range(nh):
                    g = kc_ch * 2 + h2
                    nc.gpsimd.dma_start(
                        out=ktd_sb[g][64:128, so : so + SQ],
                        in_=ktd_sb[g][0:64, so : so + SQ],
                    )

        def q_proj(c):
            m = min(128, H - c * 128)
            qps = psp.tile([128, SQ], F32, tag="st", name=f"qps{c}")
            for dc in range(NDC):
                for sb2 in range(2):
                    nc.tensor.matmul(
                        qps[:m, sb2 * 512 : (sb2 + 1) * 512],
                        lhsT=wq_sb[dc][:, c * 128 : c * 128 + m],
                        rhs=hs_sb[dc][:, QO + sb2 * 512 : QO + (sb2 + 1) * 512],
                        start=(dc == 0),
                        stop=(dc == NDC - 1),
                    )
            cast_rope(
                qps[:m], m, SQ, cosq_sb, sinq_sb, [(qt_sb[c][0:m, :], 0)], f"q{c}"
            )
            if c == NEC - 1:
                # duplicate head 8 rows for the qb-paired score matmuls
                nc.gpsimd.dma_start(out=qt_sb[c][64:128, :], in_=qt_sb[c][0:64, :])

        def v_proj(kc):
            vps = psp.tile([128, SQ], F32, tag="st", name=f"vps{kc}")
            for dc in range(NDC):
                nc.tensor.matmul(
                    vps[:, :KV],
                    lhsT=hs_sb[dc][:, kc * 128 : (kc + 1) * 128],
                    rhs=wv_sb[dc][:, :],
                    start=(dc == 0),
                    stop=(dc == NDC - 1),
                )
            nc.vector.memset(
                va_sb[kc].rearrange("p (g w) -> p g w", g=G)[:, :, 64:65], 1.0
            )
            dst = va_sb[kc].rearrange("p (g w) -> p g w", g=G)[:, :, 0:64]
            srcv = vps[:, :KV].rearrange("p (g w) -> p g w", g=G)
            nc.vector.tensor_copy(dst, srcv)

        def exp_op(at_t, st, width, kc):
            if kc in DVE_KC:
                nc.vector.tensor_scalar_add(at_t[:, :width].bitcast(I16), st[:, :width], SCHR_B)
            else:
                nc.scalar.activation(
                    at_t[:, :width],
                    st[:, :width],
                    mybir.ActivationFunctionType.Exp,
                    scale=ACT_SCALE,
                )

        def norm(h, av, qb):
            """ot[h] rows = av[0:64] * (1/av[64]) broadcast.

            Evacuate the av PSUM bank in one fast fp16 cast so the bank frees
            immediately (av pool is only double-buffered); the rest of the
            chain runs from SBUF at 16-bit DVE rates. custom-DVE ops drop
            PSUM partition offsets, so the denominator comes from the SBUF
            copy as well."""
            avc = miscp.tile([65, 512], F16, tag="avc", name=f"avc{h}{qb}")
            nc.vector.tensor_copy(avc, av)
            dn = miscp.tile([1, 512], F32, tag="dn", name=f"dn{h}{qb}")
            nc.vector.tensor_copy(dn, avc[64:65, :])
            rd = miscp.tile([1, 512], F32, tag="rd", name=f"rd{h}{qb}")
            nc.vector.reciprocal_approx_fast(out=rd, in_=dn)
            bc = miscp.tile([64, 512], F32, tag="bc", name=f"bc{h}{qb}")
            nc.gpsimd.partition_broadcast(bc, rd)
            row = (h % 2) * 64
            nc.vector.tensor_mul(
                ot_sb[h // 2][row : row + 64, qb * 512 : (qb + 1) * 512],
                avc[0:64, :],
                bc,
            )

        # ---------------- preamble projections ----------------
        # minimal preamble so the exp stream (the bottleneck engine) starts
        # as early as possible; everything else streams in as fillers.
        k_proj(0)
        q_proj(0)
        for kc in range(3):
            v_proj(kc)

        # ---------------- attention ----------------
        # filler projections interleaved into the attention loop, keyed by
        # (pair, qb, kc) -> list of thunks. They keep the PE dense while
        # ACT/DVE chew on the exp stream. Fillers with DVE-side work are
        # staggered away from the DVE-exp chunks (DVE queue is in-order).
        fillers = {}
        fillers.setdefault((0, 0, 0), []).append(lambda: k_proj(1, chunks=(0,)))
        fillers.setdefault((0, 0, 2), []).append(lambda: k_proj(1, chunks=(1,)))
        fillers.setdefault((0, 0, 1), []).append(lambda: v_proj(3))
        for kc in range(4, NKC):
            fillers.setdefault((0, 0, kc - 2), []).append(lambda kc=kc: v_proj(kc))
        fillers.setdefault((0, 1, 3), []).append(lambda: q_proj(1))
        fillers.setdefault((1, 0, 3), []).append(lambda: q_proj(2))
        fillers.setdefault((2, 0, 3), []).append(lambda: q_proj(3))
        fillers.setdefault((3, 0, 3), []).append(lambda: q_proj(4))

        for pair in range(4):
            hA = 2 * pair
            hB = hA + 1
            gA = hA // 3
            gB = hB // 3
            c = pair
            for qb in range(2):
                avA = avp.tile([65, 512], F32, tag="av", name=f"avA{pair}{qb}")
                avB = avp.tile([65, 512], F32, tag="av", name=f"avB{pair}{qb}")
                pend = None
                for kc in range(NKC):
                    for f in fillers.get((pair, qb, kc), ()):
                        f()
                    kcs = slice(kc * 128, (kc + 1) * 128)
                    qbs = slice(qb * 512, (qb + 1) * 512)
                    st = psp.tile([128, 1024], F32, tag="st", name=f"st{pair}{qb}{kc}")
                    nc.tensor.matmul(
                        st[:, 0:512],
                        lhsT=ktd_sb[gA][0:64, kcs],
                        rhs=qt_sb[c][0:64, qbs],
                        start=True,
                        stop=True,
                    )
                    nc.tensor.matmul(
                        st[:, 512:1024],
                        lhsT=ktd_sb[gB][64:128, kcs],
                        rhs=qt_sb[c][64:128, qbs],
                        start=True,
                        stop=True,
                    )
                    at_t = attnp.tile([128, 1024], F16, tag="at", name=f"at{pair}{qb}{kc}")
                    exp_op(at_t, st, 1024, kc)
                    if pend is not None:
                        pat, pkc = pend
                        nc.tensor.matmul(
                            avA,
                            lhsT=va_sb[pkc][:, gA * 65 : gA * 65 + 65],
                            rhs=pat[:, 0:512],
                            start=(pkc == 0),
                            stop=False,
                        )
                        nc.tensor.matmul(
                            avB,
                            lhsT=va_sb[pkc][:, gB * 65 : gB * 65 + 65],
                            rhs=pat[:, 512:1024],
                            start=(pkc == 0),
                            stop=False,
                        )
                    pend = (at_t, kc)
                pat, pkc = pend
                nc.tensor.matmul(
                    avA,
                    lhsT=va_sb[pkc][:, gA * 65 : gA * 65 + 65],
                    rhs=pat[:, 0:512],
                    start=False,
                    stop=True,
                )
                nc.tensor.matmul(
                    avB,
                    lhsT=va_sb[pkc][:, gB * 65 : gB * 65 + 65],
                    rhs=pat[:, 512:1024],
                    start=False,
                    stop=True,
                )
                norm(hA, avA, qb)
                norm(hB, avB, qb)

        # pair 4: single head 8, qb0/qb1 processed together (row-packed via
        # the duplicated qt rows), so it runs at the same rate as full pairs.
        g2 = 2
        av0 = avp.tile([65, 512], F32, tag="av", name="av8q0")
        av1 = avp.tile([65, 512], F32, tag="av", name="av8q1")
        pend = None
        for kc in range(NKC):
            kcs = slice(kc * 128, (kc + 1) * 128)
            st = psp.tile([128, 1024], F32, tag="st", name=f"st8{kc}")
            nc.tensor.matmul(
                st[:, 0:512],
                lhsT=ktd_sb[g2][0:64, kcs],
                rhs=qt_sb[4][0:64, 0:512],
                start=True,
                stop=True,
            )
            nc.tensor.matmul(
                st[:, 512:1024],
                lhsT=ktd_sb[g2][64:128, kcs],
                rhs=qt_sb[4][64:128, 512:1024],
                start=True,
                stop=True,
            )
            at_t = attnp.tile([128, 1024], F16, tag="at", name=f"at8{kc}")
            exp_op(at_t, st, 1024, kc)
            if pend is not None:
                pat, pkc = pend
                nc.tensor.matmul(
                    av0,
                    lhsT=va_sb[pkc][:, g2 * 65 : g2 * 65 + 65],
                    rhs=pat[:, 0:512],
                    start=(pkc == 0),
                    stop=False,
                )
                nc.tensor.matmul(
                    av1,
                    lhsT=va_sb[pkc][:, g2 * 65 : g2 * 65 + 65],
                    rhs=pat[:, 512:1024],
                    start=(pkc == 0),
                    stop=False,
                )
            pend = (at_t, kc)
        pat, pkc = pend
        nc.tensor.matmul(
            av0,
            lhsT=va_sb[pkc][:, g2 * 65 : g2 * 65 + 65],
            rhs=pat[:, 0:512],
            start=False,
            stop=True,
        )
        nc.tensor.matmul(
            av1,
            lhsT=va_sb[pkc][:, g2 * 65 : g2 * 65 + 65],
            rhs=pat[:, 512:1024],
            start=False,
            stop=True,
        )
        norm(8, av0, 0)
        norm(8, av1, 1)

        # ---------------- output projection ----------------
        for ec in range(NEC):
            m = min(128, H - ec * 128)
            for sb2 in range(2):
                ft = psp.tile([128, SQ], F32, tag="st", name=f"ft{ec}{sb2}")[:, :512]
                for cc in range(NEC):
                    k = min(128, H - cc * 128)
                    nc.tensor.matmul(
                        ft[:m, :],
                        lhsT=wo_sb[cc][:k, ec * 128 : ec * 128 + m],
                        rhs=ot_sb[cc][:k, sb2 * 512 : (sb2 + 1) * 512],
                        start=(cc == 0),
                        stop=(cc == NEC - 1),
                    )
                fts = miscp.tile([128, 512], F32, tag="fts", name=f"fts{ec}{sb2}")
                if (ec * 2 + sb2) % 2 == 0:
                    nc.scalar.copy(fts[:m, :], ft[:m, :])
                else:
                    nc.vector.tensor_copy(fts[:m, :], ft[:m, :])
                qeng[(ec * 2 + sb2) % 4].dma_start(
                    out=out[ec * 128 : ec * 128 + m, sb2 * 512 : (sb2 + 1) * 512],
                    in_=fts[:m, :],
                )


_NC_CACHE = {}


def _get_nc():
    if "nc" not in _NC_CACHE:
        _NC_CACHE["nc"] = _build_bass()
    return _NC_CACHE["nc"]


def kernel(hidden_states, wq, wk, wv, wo):
    cos4, sin2 = _rope_tables()  # fp32 [128, S]

    wq16 = np.zeros((HP, H), np.float16)
    wq16[:H] = wq.T.astype(np.float16)
    wk16 = np.zeros((HP, KV), np.float16)
    wk16[:H] = wk.T.astype(np.float16)
    wv16 = np.zeros((HP, KV), np.float16)
    wv16[:H] = wv.T.astype(np.float16)
    wo16 = wo.T.astype(np.float16)

    cosk0 = cos4.astype(np.float16)
    sink0 = sin2.astype(np.float16)
    cosk1 = np.roll(cosk0, -SQ, axis=1)
    sink1 = np.roll(sink0, -SQ, axis=1)
    # Q-side tables carry the Schraudolph scale so scores land pre-multiplied;
    # per seq-half the q columns are original positions [half*SQ, (half+1)*SQ)
    cosq0 = (cos4[:, :SQ] * SCHR_A).astype(np.float16)
    sinq0 = (sin2[:, :SQ] * SCHR_A).astype(np.float16)
    cosq1 = (cos4[:, SQ:] * SCHR_A).astype(np.float16)
    sinq1 = (sin2[:, SQ:] * SCHR_A).astype(np.float16)

    in_maps = []
    core_ids = list(range(8))
    for c in core_ids:
        b, half = c // 2, c % 2
        hsT16 = np.zeros((HP, S), np.float16)
        hsT16[:H] = hidden_states[b].T.astype(np.float16)
        if half == 1:
            # roll so this core's queries sit at columns [0, SQ); keys keep
            # their correct rope position via the equally-rolled cos/sin.
            hsT16 = np.roll(hsT16, -SQ, axis=1)
        in_maps.append(
            {
                "hsT": hsT16,
                "wqT": wq16,
                "wkT": wk16,
                "wvT": wv16,
                "woT": wo16,
                "cosq": cosq0 if half == 0 else cosq1,
                "sinq": sinq0 if half == 0 else sinq1,
                "cosk": cosk0 if half == 0 else cosk1,
                "sink": sink0 if half == 0 else sink1,
            }
        )

    global _LAST_IN_MAPS
    _LAST_IN_MAPS = in_maps
    nc = _get_nc()
    res = run_bass_kernel_spmd(nc, in_maps, core_ids=core_ids)

    out = np.empty((B, S, H), np.float32)
    for c in core_ids:
        b, half = c // 2, c % 2
        out[b, half * SQ : (half + 1) * SQ, :] = res.results[c]["o"].T
    return out


if __name__ == "__main__":
    rng = np.random.default_rng(0)
    hs = rng.standard_normal((B, S, H), dtype=np.float32)
    s = 1.0 / np.sqrt(H)
    wq = rng.standard_normal((H, H), dtype=np.float32) * s
    wk = rng.standard_normal((KV, H), dtype=np.float32) * s
    wv = rng.standard_normal((KV, H), dtype=np.float32) * s
    wo = rng.standard_normal((H, H), dtype=np.float32) * s
    o = kernel(hidden_states=hs, wq=wq, wk=wk, wv=wv, wo=wo)
    print(o.shape, o.dtype, np.abs(o).mean())


# revision 45
# speedup vs baseline: 1.0187x; 1.0187x over previous
"""Trainium2 Bass kernel for GQA attention (B=4, S=2048, H=576, 9 heads / 3 KV groups, RoPE).

Sharding: 8 cores = (batch b, seq-half) pairs. Each core computes the full
attention output for 1024 query rows of one batch element (keys/values over
the full 2048 positions of that batch element are recomputed locally; no
collectives needed).

Layout strategy: everything stays "transposed" (features on partitions, seq on
free dim):
  QT = wq @ hsT, KT = wk @ hsT (RoPE applied in T space on DVE)
  V natural [s, hv] via lhsT = hsT chunks; va layout per group = [ones | V64]
  ST[k, q] = KT.T-stationary @ QT (two heads row-tiled, concurrent in PE)
  exp: split between ACT (exact) and DVE (Schraudolph fp16 bit-hack) so both
  engines work in parallel; attnT fp16 in SBUF
  avT[1+hd, q] = [ones | V].T @ attnT  (row 0 = softmax denominator)
  final^T = woT.T-stationary @ (avT[1:65] / avT[0])
Matmul inputs fp16 (fp32 PSUM accumulation), output fp32.
"""

import sys

if "/opt/trn_rl_repo" not in sys.path:
    sys.path.insert(0, "/opt/trn_rl_repo")

import numpy as np

import concourse.bass as bass
import concourse.mybir as mybir
import concourse.tile as tile
from concourse import bacc
from concourse.bass_utils import run_bass_kernel_spmd

F16 = mybir.dt.float16
F32 = mybir.dt.float32
I16 = mybir.dt.int16

B = 4
S = 2048
SQ = 1024  # query rows per core
H = 576
HP = 640  # hidden padded to 5*128
NH = 9
HD = 64
KV = 192
G = 3
ROPE_THETA = 10000.0
SCALE = 1.0 / 8.0  # 1/sqrt(HD)

NDC = HP // 128  # 5 contraction chunks
NEC = 5  # output feature chunks of QT (4*128 + 64)
NKC = S // 128  # 16 key chunks

# --- exp split: which key chunks use the DVE Schraudolph approx-exp ---------
# exp(x) ~ bitcast_f16(int16(round(x*SCHR_A + SCHR_B)));  x = raw score, the
# 1/8 softmax scale is folded into SCHR_A. ~2-3%% relative error, zero-mean-ish
# component cancels in the softmax ratio; validated end-to-end vs tolerance.
# SCHR_A is folded into the Q-side rope tables on the host, so scores arrive
# in PSUM already scaled: the DVE op is a single scalar add (+SCHR_B) and the
# ACT path just uses scale=ln2/1024 instead of 1/8.
DVE_KC = (2, 6, 10, 14)
SCHR_A = (1024.0 / float(np.log(2.0))) * SCALE
SCHR_B = 15360.0 - 44.0
ACT_SCALE = float(np.log(2.0)) / 1024.0


def _rope_tables():
    """fp32 master cos/sin tables [128, S] with dest-indexed sin signs."""
    inv_freq = 1.0 / (ROPE_THETA ** (np.arange(0, HD, 2, dtype=np.float32) / HD))
    t = np.arange(S, dtype=np.float32)
    freqs = np.einsum("i,j->ij", inv_freq, t)  # [32, S]
    cos32 = np.cos(freqs)
    sin32 = np.sin(freqs)
    cos4 = np.tile(cos32, (4, 1))  # [128, S]
    # sin indexed by DEST rows: out[j<32] = q[j]*cos - q[j+32]*sin[j];
    # out[j>=32] = q[j]*cos + q[j-32]*sin. The shifted tile sh[j] holds the
    # cross row, so sign pattern per 64-block is [-sin32; +sin32].
    sin2 = np.concatenate([-sin32, sin32, -sin32, sin32], axis=0)  # [128, S]
    return cos4, sin2


def _build_bass():
    nc = bacc.Bacc("TRN2", target_bir_lowering=False)

    hsT = nc.declare_dram_parameter("hsT", [HP, S], F16, isOutput=False)
    wqT = nc.declare_dram_parameter("wqT", [HP, H], F16, isOutput=False)
    wkT = nc.declare_dram_parameter("wkT", [HP, KV], F16, isOutput=False)
    wvT = nc.declare_dram_parameter("wvT", [HP, KV], F16, isOutput=False)
    woT = nc.declare_dram_parameter("woT", [H, H], F16, isOutput=False)
    cosq = nc.declare_dram_parameter("cosq", [128, SQ], F16, isOutput=False)
    sinq = nc.declare_dram_parameter("sinq", [128, SQ], F16, isOutput=False)
    cosk = nc.declare_dram_parameter("cosk", [128, S], F16, isOutput=False)
    sink = nc.declare_dram_parameter("sink", [128, S], F16, isOutput=False)
    out = nc.declare_dram_parameter("o", [H, SQ], F32, isOutput=True)

    with tile.TileContext(nc) as tc:
        kernel_body(nc, tc, hsT, wqT, wkT, wvT, woT, cosq, sinq, cosk, sink, out)

    nc.compile()
    return nc


def kernel_body(nc, tc, hsT, wqT, wkT, wvT, woT, cosq, sinq, cosk, sink, out):
    import contextlib

    ctx = contextlib.ExitStack()
    with ctx:
        # ---------------- persistent SBUF pools ----------------
        wpool = ctx.enter_context(tc.tile_pool(name="w", bufs=1))
        qtp = ctx.enter_context(tc.tile_pool(name="qt", bufs=1))
        ktp = ctx.enter_context(tc.tile_pool(name="kt", bufs=1))
        vap = ctx.enter_context(tc.tile_pool(name="va", bufs=1))
        otp = ctx.enter_context(tc.tile_pool(name="ot", bufs=1))
        ropep = ctx.enter_context(tc.tile_pool(name="rope", bufs=2))
        attnp = ctx.enter_context(tc.tile_pool(name="attn", bufs=4))
        miscp = ctx.enter_context(tc.tile_pool(name="misc", bufs=4))

        # ---------------- load inputs to SBUF ----------------
        hs_sb = []
        wq_sb = []
        wk_sb = []
        wv_sb = []
        wo_sb = []
        # spread the ~5MB of input loads across per-engine HWDGE queues so
        # they run in parallel instead of serializing on the sync queue
        qeng = [nc.sync, nc.scalar, nc.sync, nc.scalar]
        for dc in range(NDC):
            t = wpool.tile([128, S], F16, tag=f"hs{dc}", name=f"hs{dc}")
            qeng[dc % 4].dma_start(out=t, in_=hsT[dc * 128 : (dc + 1) * 128, :])
            hs_sb.append(t)
            t = wpool.tile([128, H], F16, tag=f"wq{dc}", name=f"wq{dc}")
            qeng[(dc + 1) % 4].dma_start(out=t, in_=wqT[dc * 128 : (dc + 1) * 128, :])
            wq_sb.append(t)
            t = wpool.tile([128, KV], F16, tag=f"wk{dc}", name=f"wk{dc}")
            qeng[(dc + 2) % 4].dma_start(out=t, in_=wkT[dc * 128 : (dc + 1) * 128, :])
            wk_sb.append(t)
            t = wpool.tile([128, KV], F16, tag=f"wv{dc}", name=f"wv{dc}")
            qeng[(dc + 3) % 4].dma_start(out=t, in_=wvT[dc * 128 : (dc + 1) * 128, :])
            wv_sb.append(t)
        for ec in range(NEC):
            m = min(128, H - ec * 128)
            t = wpool.tile([128, H], F16, tag=f"wo{ec}", name=f"wo{ec}")
            qeng[ec % 4].dma_start(out=t[:m, :], in_=woT[ec * 128 : ec * 128 + m, :])
            wo_sb.append(t)
        cosq_sb = wpool.tile([128, SQ], F16, tag="cosq")
        nc.scalar.dma_start(out=cosq_sb, in_=cosq[:, :])
        sinq_sb = wpool.tile([128, SQ], F16, tag="sinq")
        nc.gpsimd.dma_start(out=sinq_sb, in_=sinq[:, :])
        cosk_sb = wpool.tile([128, S], F16, tag="cosk")
        nc.sync.dma_start(out=cosk_sb, in_=cosk[:, :])
        sink_sb = wpool.tile([128, S], F16, tag="sink")
        nc.sync.dma_start(out=sink_sb, in_=sink[:, :])

        # persistent activation tensors
        qt_sb = [qtp.tile([128, SQ], F16, tag=f"qt{c}", name=f"qt{c}") for c in range(NEC)]
        ktd_sb = [ktp.tile([128, S], F16, tag=f"ktd{g}", name=f"ktd{g}") for g in range(G)]
        va_sb = [vap.tile([128, 3 * 65], F16, tag=f"va{kc}", name=f"va{kc}") for kc in range(NKC)]
        ot_sb = [otp.tile([128, SQ], F16, tag=f"ot{c}", name=f"ot{c}") for c in range(NEC)]

        # PSUM pools: st [128,1024] x3 = 6 banks, av [65,512] x2 = 2 banks
        psp = ctx.enter_context(tc.tile_pool(name="ps", bufs=3, space="PSUM"))
        avp = ctx.enter_context(tc.tile_pool(name="avp", bufs=2, space="PSUM"))

        def cast_rope(ps_ap, nrows, width, cos_ap, sin_ap, dst_writes, nm):
            """cast psum->sbuf fp16, then rope via DMA half-swap + 3 DVE ops.

            dst_writes: list of (dst_ap [64 or 128 rows, width], src_row)."""
            raw = ropep.tile([128, width], F16, tag="rraw", name=f"rr{nm}")
            nc.vector.tensor_copy(raw[:nrows], ps_ap)
            sh = ropep.tile([128, width], F16, tag="rsh", name=f"rs{nm}")
            for b0 in range(0, nrows, 64):
                nc.gpsimd.dma_start(out=sh[b0 : b0 + 32], in_=raw[b0 + 32 : b0 + 64])
                nc.gpsimd.dma_start(out=sh[b0 + 32 : b0 + 64], in_=raw[b0 : b0 + 32])
            t1 = ropep.tile([128, width], F16, tag="rt1", name=f"r1{nm}")
            t2 = ropep.tile([128, width], F16, tag="rt2", name=f"r2{nm}")
            nc.vector.tensor_mul(t1[:nrows], raw[:nrows], cos_ap[:nrows])
            nc.vector.tensor_mul(t2[:nrows], sh[:nrows], sin_ap[:nrows])
            for dst, row in dst_writes:
                n = dst.partition_size()
                nc.vector.tensor_add(dst, t1[row : row + n], t2[row : row + n])

        # Queries are always hsT columns [0, SQ): cores covering the second
        # seq half pass hsT (and cos/sin) rolled by -SQ columns.
        QO = 0

        def k_proj(piece, chunks=(0, 1)):
            so = piece * SQ
            for kc_ch, (roff, nh) in enumerate([(0, 2), (128, 1)]):
                if kc_ch not in chunks:
                    continue
                m = nh * 64
                kps = psp.tile([128, SQ], F32, tag="st", name=f"kps{piece}{kc_ch}")
                for dc in range(NDC):
                    for sb2 in range(2):
                        nc.tensor.matmul(
                            kps[:m, sb2 * 512 : (sb2 + 1) * 512],
                            lhsT=wk_sb[dc][:, roff : roff + m],
                            rhs=hs_sb[dc][:, so + sb2 * 512 : so + (sb2 + 1) * 512],
                            start=(dc == 0),
                            stop=(dc == NDC - 1),
                        )
                writes = []
                for h2 in range(nh):
                    g = kc_ch * 2 + h2
                    writes.append((ktd_sb[g][0:64, so : so + SQ], h2 * 64))
                cast_rope(
                    kps[:m],
                    m,
                    SQ,
                    cosk_sb[:, so : so + SQ],
                    sink_sb[:, so : so + SQ],
                    writes,
                    f"k{piece}{kc_ch}",
                )
                # duplicate rows 0-63 -> 64-127 for row-packed score matmuls
                for h2 in range(nh):
                    g = kc_ch * 2 + h2
                    nc.gpsimd.dma_start(
                        out=ktd_sb[g][64:128, so : so + SQ],
                        in_=ktd_sb[g][0:64, so : so + SQ],
                    )

        def q_proj(c):
            m = min(128, H - c * 128)
            qps = psp.tile([128, SQ], F32, tag="st", name=f"qps{c}")
            for dc in range(NDC):
                for sb2 in range(2):
                    nc.tensor.matmul(
                        qps[:m, sb2 * 512 : (sb2 + 1) * 512],
                        lhsT=wq_sb[dc][:, c * 128 : c * 128 + m],
                        rhs=hs_sb[dc][:, QO + sb2 * 512 : QO + (sb2 + 1) * 512],
                        start=(dc == 0),
                        stop=(dc == NDC - 1),
                    )
            cast_rope(
                qps[:m], m, SQ, cosq_sb, sinq_sb, [(qt_sb[c][0:m, :], 0)], f"q{c}"
            )
            if c == NEC - 1:
                # duplicate head 8 rows for the qb-paired score matmuls
                nc.gpsimd.dma_start(out=qt_sb[c][64:128, :], in_=qt_sb[c][0:64, :])

        def v_proj(kc):
            vps = psp.tile([128, SQ], F32, tag="st", name=f"vps{kc}")
            for dc in range(NDC):
                nc.tensor.matmul(
                    vps[:, :KV],
                    lhsT=hs_sb[dc][:, kc * 128 : (kc + 1) * 128],
                    rhs=wv_sb[dc][:, :],
                    start=(dc == 0),
                    stop=(dc == NDC - 1),
                )
            nc.vector.memset(
                va_sb[kc].rearrange("p (g w) -> p g w", g=G)[:, :, 64:65], 1.0
            )
            dst = va_sb[kc].rearrange("p (g w) -> p g w", g=G)[:, :, 0:64]
            srcv = vps[:, :KV].rearrange("p (g w) -> p g w", g=G)
            nc.vector.tensor_copy(dst, srcv)

        def exp_op(at_t, st, width, kc):
            if kc in DVE_KC:
                nc.vector.tensor_scalar_add(at_t[:, :width].bitcast(I16), st[:, :width], SCHR_B)
            else:
                nc.scalar.activation(
                    at_t[:, :width],
                    st[:, :width],
                    mybir.ActivationFunctionType.Exp,
                    scale=ACT_SCALE,
                )

        def norm(h, av, qb):
            """ot[h] rows = av[0:64] * (1/av[64]) broadcast.

            Evacuate the av PSUM bank in one fast fp16 cast so the bank frees
            immediately (av pool is only double-buffered); the rest of the
            chain runs from SBUF at 16-bit DVE rates. custom-DVE ops drop
            PSUM partition offsets, so the denominator comes from the SBUF
            copy as well."""
            avc = miscp.tile([65, 512], F16, tag="avc", name=f"avc{h}{qb}")
            nc.vector.tensor_copy(avc, av)
            dn = miscp.tile([1, 512], F32, tag="dn", name=f"dn{h}{qb}")
            nc.vector.tensor_copy(dn, avc[64:65, :])
            rd = miscp.tile([1, 512], F32, tag="rd", name=f"rd{h}{qb}")
            nc.vector.reciprocal_approx_fast(out=rd, in_=dn)
            bc = miscp.tile([64, 512], F32, tag="bc", name=f"bc{h}{qb}")
            nc.gpsimd.partition_broadcast(bc, rd)
            row = (h % 2) * 64
            nc.vector.tensor_mul(
                ot_sb[h // 2][row : row + 64, qb * 512 : (qb + 1) * 512],
                avc[0:64, :],
                bc,
            )

        # ---------------- preamble projections ----------------
        # minimal preamble so the exp stream (the bottleneck engine) starts
        # as early as possible; everything else streams in as fillers.
        k_proj(0)
        q_proj(0)
        v_proj(0)
        v_proj(1)

        # ---------------- attention ----------------
        # filler projections interleaved into the attention loop, keyed by
        # (pair, qb, kc) -> list of thunks. They keep the PE dense while
        # ACT/DVE chew on the exp stream. Fillers with DVE-side work are
        # staggered away from the DVE-exp chunks (DVE queue is in-order).
        fillers = {}
        fillers.setdefault((0, 0, 0), []).append(lambda: k_proj(1, chunks=(0,)))
        fillers.setdefault((0, 0, 1), []).append(lambda: v_proj(2))
        fillers.setdefault((0, 0, 1), []).append(lambda: v_proj(3))
        fillers.setdefault((0, 0, 3), []).append(lambda: k_proj(1, chunks=(1,)))
        for kc in range(4, NKC):
            fillers.setdefault((0, 0, kc - 2), []).append(lambda kc=kc: v_proj(kc))
        fillers.setdefault((0, 1, 3), []).append(lambda: q_proj(1))
        fillers.setdefault((1, 0, 3), []).append(lambda: q_proj(2))
        fillers.setdefault((2, 0, 3), []).append(lambda: q_proj(3))
        fillers.setdefault((3, 0, 3), []).append(lambda: q_proj(4))

        for pair in range(4):
            hA = 2 * pair
            hB = hA + 1
            gA = hA // 3
            gB = hB // 3
            c = pair
            for qb in range(2):
                avA = avp.tile([65, 512], F32, tag="av", name=f"avA{pair}{qb}")
                avB = avp.tile([65, 512], F32, tag="av", name=f"avB{pair}{qb}")
                pend = None
                for kc in range(NKC):
                    for f in fillers.get((pair, qb, kc), ()):
                        f()
                    kcs = slice(kc * 128, (kc + 1) * 128)
                    qbs = slice(qb * 512, (qb + 1) * 512)
                    st = psp.tile([128, 1024], F32, tag="st", name=f"st{pair}{qb}{kc}")
                    nc.tensor.matmul(
                        st[:, 0:512],
                        lhsT=ktd_sb[gA][0:64, kcs],
                        rhs=qt_sb[c][0:64, qbs],
                        start=True,
                        stop=True,
                    )
                    nc.tensor.matmul(
                        st[:, 512:1024],
                        lhsT=ktd_sb[gB][64:128, kcs],
                        rhs=qt_sb[c][64:128, qbs],
                        start=True,
                        stop=True,
                    )
                    at_t = attnp.tile([128, 1024], F16, tag="at", name=f"at{pair}{qb}{kc}")
                    exp_op(at_t, st, 1024, kc)
                    if pend is not None:
                        pat, pkc = pend
                        nc.tensor.matmul(
                            avA,
                            lhsT=va_sb[pkc][:, gA * 65 : gA * 65 + 65],
                            rhs=pat[:, 0:512],
                            start=(pkc == 0),
                            stop=False,
                        )
                        nc.tensor.matmul(
                            avB,
                            lhsT=va_sb[pkc][:, gB * 65 : gB * 65 + 65],
                            rhs=pat[:, 512:1024],
                            start=(pkc == 0),
                            stop=False,
                        )
                    pend = (at_t, kc)
                pat, pkc = pend
                nc.tensor.matmul(
                    avA,
                    lhsT=va_sb[pkc][:, gA * 65 : gA * 65 + 65],
                    rhs=pat[:, 0:512],
                    start=False,
                    stop=True,
                )
                nc.tensor.matmul(
                    avB,
                    lhsT=va_sb[pkc][:, gB * 65 : gB * 65 + 65],
                    rhs=pat[:, 512:1024],
                    start=False,
                    stop=True,
                )
                norm(hA, avA, qb)
                norm(hB, avB, qb)

        # pair 4: single head 8, qb0/qb1 processed together (row-packed via
        # the duplicated qt rows), so it runs at the same rate as full pairs.
        g2 = 2
        av0 = avp.tile([65, 512], F32, tag="av", name="av8q0")
        av1 = avp.tile([65, 512], F32, tag="av", name="av8q1")
        pend = None
        for kc in range(NKC):
            kcs = slice(kc * 128, (kc + 1) * 128)
            st = psp.tile([128, 1024], F32, tag="st", name=f"st8{kc}")
            nc.tensor.matmul(
                st[:, 0:512],
                lhsT=ktd_sb[g2][0:64, kcs],
                rhs=qt_sb[4][0:64, 0:512],
                start=True,
                stop=True,
            )
            nc.tensor.matmul(
                st[:, 512:1024],
                lhsT=ktd_sb[g2][64:128, kcs],
                rhs=qt_sb[4][64:128, 512:1024],
                start=True,
                stop=True,
            )
            at_t = attnp.tile([128, 1024], F16, tag="at", name=f"at8{kc}")
            exp_op(at_t, st, 1024, kc)
            if pend is not None:
                pat, pkc = pend
                nc.tensor.matmul(
                    av0,
                    lhsT=va_sb[pkc][:, g2 * 65 : g2 * 65 + 65],
                    rhs=pat[:, 0:512],
                    start=(pkc == 0),
                    stop=False,
                )
                nc.tensor.matmul(
                    av1,
                    lhsT=va_sb[pkc][:, g2 * 65 : g2 * 65 + 65],
                    rhs=pat[:, 512:1024],
                    start=(pkc == 0),
                    stop=False,
                )
            pend = (at_t, kc)
        pat, pkc = pend
        nc.tensor.matmul(
            av0,
            lhsT=va_sb[pkc][:, g2 * 65 : g2 * 65 + 65],
            rhs=pat[:, 0:512],
            start=False,
            stop=True,
        )
        nc.tensor.matmul(
            av1,
            lhsT=va_sb[pkc][:, g2 * 65 : g2 * 65 + 65],
            rhs=pat[:, 512:1024],
            start=False,
            stop=True,
        )
        norm(8, av0, 0)
        norm(8, av1, 1)

        # ---------------- output projection ----------------
        for ec in range(NEC):
            m = min(128, H - ec * 128)
            for sb2 in range(2):
                ft = psp.tile([128, SQ], F32, tag="st", name=f"ft{ec}{sb2}")[:, :512]
                for cc in range(NEC):
                    k = min(128, H - cc * 128)
                    nc.tensor.matmul(
                        ft[:m, :],
                        lhsT=wo_sb[cc][:k, ec * 128 : ec * 128 + m],
                        rhs=ot_sb[cc][:k, sb2 * 512 : (sb2 + 1) * 512],
                        start=(cc == 0),
                        stop=(cc == NEC - 1),
                    )
                fts = miscp.tile([128, 512], F32, tag="fts", name=f"fts{ec}{sb2}")
                if (ec * 2 + sb2) % 2 == 0:
                    nc.scalar.copy(fts[:m, :], ft[:m, :])
                else:
                    nc.vector.tensor_copy(fts[:m, :], ft[:m, :])
                qeng[(ec * 2 + sb2) % 4].dma_start(
                    out=out[ec * 128 : ec * 128 + m, sb2 * 512 : (sb2 + 1) * 512],
                    in_=fts[:m, :],
                )


_NC_CACHE = {}


def _get_nc():
    if "nc" not in _NC_CACHE:
        _NC_CACHE["nc"] = _build_bass()
    return _NC_CACHE["nc"]


def kernel(hidden_states, wq, wk, wv, wo):
    cos4, sin2 = _rope_tables()  # fp32 [128, S]

    wq16 = np.zeros((HP, H), np.float16)
    wq16[:H] = wq.T.astype(np.float16)
    wk16 = np.zeros((HP, KV), np.float16)
    wk16[:H] = wk.T.astype(np.float16)
    wv16 = np.zeros((HP, KV), np.float16)
    wv16[:H] = wv.T.astype(np.float16)
    wo16 = wo.T.astype(np.float16)

    cosk0 = cos4.astype(np.float16)
    sink0 = sin2.astype(np.float16)
    cosk1 = np.roll(cosk0, -SQ, axis=1)
    sink1 = np.roll(sink0, -SQ, axis=1)
    # Q-side tables carry the Schraudolph scale so scores land pre-multiplied;
    # per seq-half the q columns are original positions [half*SQ, (half+1)*SQ)
    cosq0 = (cos4[:, :SQ] * SCHR_A).astype(np.float16)
    sinq0 = (sin2[:, :SQ] * SCHR_A).astype(np.float16)
    cosq1 = (cos4[:, SQ:] * SCHR_A).astype(np.float16)
    sinq1 = (sin2[:, SQ:] * SCHR_A).astype(np.float16)

    in_maps = []
    core_ids = list(range(8))
    for c in core_ids:
        b, half = c // 2, c % 2
        hsT16 = np.zeros((HP, S), np.float16)
        hsT16[:H] = hidden_states[b].T.astype(np.float16)
        if half == 1:
            # roll so this core's queries sit at columns [0, SQ); keys keep
            # their correct rope position via the equally-rolled cos/sin.
            hsT16 = np.roll(hsT16, -SQ, axis=1)
        in_maps.append(
            {
                "hsT": hsT16,
                "wqT": wq16,
                "wkT": wk16,
                "wvT": wv16,
                "woT": wo16,
                "cosq": cosq0 if half == 0 else cosq1,
                "sinq": sinq0 if half == 0 else sinq1,
                "cosk": cosk0 if half == 0 else cosk1,
                "sink": sink0 if half == 0 else sink1,
            }
        )

    global _LAST_IN_MAPS
    _LAST_IN_MAPS = in_maps
    nc = _get_nc()
    res = run_bass_kernel_spmd(nc, in_maps, core_ids=core_ids)

    out = np.empty((B, S, H), np.float32)
    for c in core_ids:
        b, half = c // 2, c % 2
        out[b, half * SQ : (half + 1) * SQ, :] = res.results[c]["o"].T
    return out


if __name__ == "__main__":
    rng = np.random.default_rng(0)
    hs = rng.standard_normal((B, S, H), dtype=np.float32)
    s = 1.0 / np.sqrt(H)
    wq = rng.standard_normal((H, H), dtype=np.float32) * s
    wk = rng.standard_normal((KV, H), dtype=np.float32) * s
    wv = rng.standard_normal((KV, H), dtype=np.float32) * s
    wo = rng.standard_normal((H, H), dtype=np.float32) * s
    o = kernel(hidden_states=hs, wq=wq, wk=wk, wv=wv, wo=wo)
    print(o.shape, o.dtype, np.abs(o).mean())


# revision 52
# speedup vs baseline: 1.1686x; 1.1471x over previous
"""Trainium2 Bass kernel for GQA attention (B=4, S=2048, H=576, 9 heads / 3 KV groups, RoPE).

Sharding: 8 cores = (batch b, seq-half) pairs. Each core computes the full
attention output for 1024 query rows of one batch element (keys/values over
the full 2048 positions of that batch element are recomputed locally; no
collectives needed).

Layout strategy: everything stays "transposed" (features on partitions, seq on
free dim):
  QT = wq @ hsT, KT = wk @ hsT (RoPE applied in T space on DVE)
  V natural [s, hv] via lhsT = hsT chunks; va layout per group = [ones | V64]
  ST[k, q] = KT.T-stationary @ QT (two heads row-tiled, concurrent in PE)
  exp: split between ACT (exact) and DVE (Schraudolph fp16 bit-hack) so both
  engines work in parallel; attnT fp16 in SBUF
  avT[1+hd, q] = [ones | V].T @ attnT  (row 0 = softmax denominator)
  final^T = woT.T-stationary @ (avT[1:65] / avT[0])
Matmul inputs fp16 (fp32 PSUM accumulation), output fp32.
"""

import sys

if "/opt/trn_rl_repo" not in sys.path:
    sys.path.insert(0, "/opt/trn_rl_repo")

import numpy as np

import concourse.bass as bass
import concourse.mybir as mybir
import concourse.tile as tile
from concourse import bacc
from concourse.bass_utils import run_bass_kernel_spmd

F16 = mybir.dt.float16
F32 = mybir.dt.float32
I16 = mybir.dt.int16

B = 4
S = 2048
SQ = 1024  # query rows per core
H = 576
HP = 640  # hidden padded to 5*128
NH = 9
HD = 64
KV = 192
G = 3
ROPE_THETA = 10000.0
SCALE = 1.0 / 8.0  # 1/sqrt(HD)

NDC = HP // 128  # 5 contraction chunks
NEC = 5  # output feature chunks of QT (4*128 + 64)
NKC = S // 128  # 16 key chunks

# --- exp split: which key chunks use the DVE Schraudolph approx-exp ---------
# exp(x) ~ bitcast_f16(int16(round(x*SCHR_A + SCHR_B)));  x = raw score, the
# 1/8 softmax scale is folded into SCHR_A. ~2-3%% relative error, zero-mean-ish
# component cancels in the softmax ratio; validated end-to-end vs tolerance.
# SCHR_A is folded into the Q-side rope tables on the host, so scores arrive
# in PSUM already scaled: the DVE op is a single scalar add (+SCHR_B) and the
# ACT path just uses scale=ln2/1024 instead of 1/8.
DVE_KC = (2, 6, 10, 14)
SCHR_A = (1024.0 / float(np.log(2.0))) * SCALE
SCHR_B = 15360.0 - 44.0
ACT_SCALE = float(np.log(2.0)) / 1024.0


def _rope_tables():
    """fp32 master cos/sin tables [128, S] with dest-indexed sin signs."""
    inv_freq = 1.0 / (ROPE_THETA ** (np.arange(0, HD, 2, dtype=np.float32) / HD))
    t = np.arange(S, dtype=np.float32)
    freqs = np.einsum("i,j->ij", inv_freq, t)  # [32, S]
    cos32 = np.cos(freqs)
    sin32 = np.sin(freqs)
    cos4 = np.tile(cos32, (4, 1))  # [128, S]
    # sin indexed by DEST rows: out[j<32] = q[j]*cos - q[j+32]*sin[j];
    # out[j>=32] = q[j]*cos + q[j-32]*sin. The shifted tile sh[j] holds the
    # cross row, so sign pattern per 64-block is [-sin32; +sin32].
    sin2 = np.concatenate([-sin32, sin32, -sin32, sin32], axis=0)  # [128, S]
    return cos4, sin2


def _build_bass():
    nc = bacc.Bacc("TRN2", target_bir_lowering=False)

    hsT = nc.declare_dram_parameter("hsT", [HP, S], F16, isOutput=False)
    wqT = nc.declare_dram_parameter("wqT", [HP, H], F16, isOutput=False)
    wkT = nc.declare_dram_parameter("wkT", [HP, KV], F16, isOutput=False)
    wvT = nc.declare_dram_parameter("wvT", [HP, KV], F16, isOutput=False)
    woT = nc.declare_dram_parameter("woT", [H, H], F16, isOutput=False)
    cosq = nc.declare_dram_parameter("cosq", [128, SQ], F16, isOutput=False)
    sinq = nc.declare_dram_parameter("sinq", [128, SQ], F16, isOutput=False)
    cosk = nc.declare_dram_parameter("cosk", [128, S], F16, isOutput=False)
    sink = nc.declare_dram_parameter("sink", [128, S], F16, isOutput=False)
    out = nc.declare_dram_parameter("o", [H, SQ], F32, isOutput=True)

    with tile.TileContext(nc) as tc:
        kernel_body(nc, tc, hsT, wqT, wkT, wvT, woT, cosq, sinq, cosk, sink, out)

    nc.compile()
    return nc


def kernel_body(nc, tc, hsT, wqT, wkT, wvT, woT, cosq, sinq, cosk, sink, out):
    import contextlib

    ctx = contextlib.ExitStack()
    with ctx:
        # ---------------- persistent SBUF pools ----------------
        wpool = ctx.enter_context(tc.tile_pool(name="w", bufs=1))
        qtp = ctx.enter_context(tc.tile_pool(name="qt", bufs=1))
        ktp = ctx.enter_context(tc.tile_pool(name="kt", bufs=1))
        vap = ctx.enter_context(tc.tile_pool(name="va", bufs=1))
        otp = ctx.enter_context(tc.tile_pool(name="ot", bufs=1))
        ropep = ctx.enter_context(tc.tile_pool(name="rope", bufs=2))
        attnp = ctx.enter_context(tc.tile_pool(name="attn", bufs=4))
        miscp = ctx.enter_context(tc.tile_pool(name="misc", bufs=4))

        # ---------------- load inputs to SBUF ----------------
        hs_sb = []
        wq_sb = []
        wk_sb = []
        wv_sb = []
        wo_sb = []
        # spread the ~5MB of input loads across per-engine HWDGE queues so
        # they run in parallel instead of serializing on the sync queue
        qeng = [nc.sync, nc.scalar, nc.sync, nc.scalar]
        for dc in range(NDC):
            t = wpool.tile([128, S], F16, tag=f"hs{dc}", name=f"hs{dc}")
            qeng[dc % 4].dma_start(out=t, in_=hsT[dc * 128 : (dc + 1) * 128, :])
            hs_sb.append(t)
            t = wpool.tile([128, H], F16, tag=f"wq{dc}", name=f"wq{dc}")
            qeng[(dc + 1) % 4].dma_start(out=t, in_=wqT[dc * 128 : (dc + 1) * 128, :])
            wq_sb.append(t)
            t = wpool.tile([128, KV], F16, tag=f"wk{dc}", name=f"wk{dc}")
            qeng[(dc + 2) % 4].dma_start(out=t, in_=wkT[dc * 128 : (dc + 1) * 128, :])
            wk_sb.append(t)
            t = wpool.tile([128, KV], F16, tag=f"wv{dc}", name=f"wv{dc}")
            qeng[(dc + 3) % 4].dma_start(out=t, in_=wvT[dc * 128 : (dc + 1) * 128, :])
            wv_sb.append(t)
        for ec in range(NEC):
            m = min(128, H - ec * 128)
            t = wpool.tile([128, H], F16, tag=f"wo{ec}", name=f"wo{ec}")
            qeng[ec % 4].dma_start(out=t[:m, :], in_=woT[ec * 128 : ec * 128 + m, :])
            wo_sb.append(t)
        cosq_sb = wpool.tile([128, SQ], F16, tag="cosq")
        nc.scalar.dma_start(out=cosq_sb, in_=cosq[:, :])
        sinq_sb = wpool.tile([128, SQ], F16, tag="sinq")
        nc.gpsimd.dma_start(out=sinq_sb, in_=sinq[:, :])
        cosk_sb = wpool.tile([128, S], F16, tag="cosk")
        nc.sync.dma_start(out=cosk_sb, in_=cosk[:, :])
        sink_sb = wpool.tile([128, S], F16, tag="sink")
        nc.sync.dma_start(out=sink_sb, in_=sink[:, :])

        # persistent activation tensors
        qt_sb = [qtp.tile([128, SQ], F16, tag=f"qt{c}", name=f"qt{c}") for c in range(NEC)]
        ktd_sb = [ktp.tile([128, S], F16, tag=f"ktd{g}", name=f"ktd{g}") for g in range(G)]
        va_sb = [vap.tile([128, 3 * 65], F16, tag=f"va{kc}", name=f"va{kc}") for kc in range(NKC)]
        ot_sb = [otp.tile([128, SQ], F16, tag=f"ot{c}", name=f"ot{c}") for c in range(NEC)]

        # PSUM pools: st [128,1024] x3 = 6 banks, av [65,512] x2 = 2 banks
        psp = ctx.enter_context(tc.tile_pool(name="ps", bufs=3, space="PSUM"))
        avp = ctx.enter_context(tc.tile_pool(name="avp", bufs=2, space="PSUM"))

        # round-robin small SBUF->SBUF DMAs across the three DMA-capable
        # queues so rope shifts / dups don't serialize on one queue
        _rr = [0]
        _rreng = [nc.gpsimd, nc.sync, nc.scalar]

        def rr_dma(out_ap, in_ap):
            _rreng[_rr[0] % 3].dma_start(out=out_ap, in_=in_ap)
            _rr[0] += 1

        def cast_rope(ps_ap, nrows, width, cos_ap, sin_ap, dst_writes, nm):
            """cast psum->sbuf fp16, then rope via DMA half-swap + 3 DVE ops.

            dst_writes: list of (dst_ap [64 or 128 rows, width], src_row)."""
            raw = ropep.tile([128, width], F16, tag="rraw", name=f"rr{nm}")
            nc.vector.tensor_copy(raw[:nrows], ps_ap)
            sh = ropep.tile([128, width], F16, tag="rsh", name=f"rs{nm}")
            for b0 in range(0, nrows, 64):
                rr_dma(sh[b0 : b0 + 32], raw[b0 + 32 : b0 + 64])
                rr_dma(sh[b0 + 32 : b0 + 64], raw[b0 : b0 + 32])
            t1 = ropep.tile([128, width], F16, tag="rt1", name=f"r1{nm}")
            t2 = ropep.tile([128, width], F16, tag="rt2", name=f"r2{nm}")
            nc.vector.tensor_mul(t1[:nrows], raw[:nrows], cos_ap[:nrows])
            nc.vector.tensor_mul(t2[:nrows], sh[:nrows], sin_ap[:nrows])
            for dst, row in dst_writes:
                n = dst.partition_size()
                nc.vector.tensor_add(dst, t1[row : row + n], t2[row : row + n])

        # Queries are always hsT columns [0, SQ): cores covering the second
        # seq half pass hsT (and cos/sin) rolled by -SQ columns.
        QO = 0

        def k_proj(piece, chunks=(0, 1)):
            so = piece * SQ
            for kc_ch, (roff, nh) in enumerate([(0, 2), (128, 1)]):
                if kc_ch not in chunks:
                    continue
                m = nh * 64
                kps = psp.tile([128, SQ], F32, tag="st", name=f"kps{piece}{kc_ch}")
                for dc in range(NDC):
                    for sb2 in range(2):
                        nc.tensor.matmul(
                            kps[:m, sb2 * 512 : (sb2 + 1) * 512],
                            lhsT=wk_sb[dc][:, roff : roff + m],
                            rhs=hs_sb[dc][:, so + sb2 * 512 : so + (sb2 + 1) * 512],
                            start=(dc == 0),
                            stop=(dc == NDC - 1),
                        )
                writes = []
                for h2 in range(nh):
                    g = kc_ch * 2 + h2
                    writes.append((ktd_sb[g][0:64, so : so + SQ], h2 * 64))
                cast_rope(
                    kps[:m],
                    m,
                    SQ,
                    cosk_sb[:, so : so + SQ],
                    sink_sb[:, so : so + SQ],
                    writes,
                    f"k{piece}{kc_ch}",
                )
                # duplicate rows 0-63 -> 64-127 for row-packed score matmuls
                for h2 in range(nh):
                    g = kc_ch * 2 + h2
                    rr_dma(
                        ktd_sb[g][64:128, so : so + SQ],
                        ktd_sb[g][0:64, so : so + SQ],
                    )

        def q_proj(c):
            m = min(128, H - c * 128)
            qps = psp.tile([128, SQ], F32, tag="st", name=f"qps{c}")
            for dc in range(NDC):
                for sb2 in range(2):
                    nc.tensor.matmul(
                        qps[:m, sb2 * 512 : (sb2 + 1) * 512],
                        lhsT=wq_sb[dc][:, c * 128 : c * 128 + m],
                        rhs=hs_sb[dc][:, QO + sb2 * 512 : QO + (sb2 + 1) * 512],
                        start=(dc == 0),
                        stop=(dc == NDC - 1),
                    )
            cast_rope(
                qps[:m], m, SQ, cosq_sb, sinq_sb, [(qt_sb[c][0:m, :], 0)], f"q{c}"
            )
            if c == NEC - 1:
                # duplicate head 8 rows for the qb-paired score matmuls
                rr_dma(qt_sb[c][64:128, :], qt_sb[c][0:64, :])

        def v_proj(kc):
            vps = psp.tile([128, SQ], F32, tag="st", name=f"vps{kc}")
            for dc in range(NDC):
                nc.tensor.matmul(
                    vps[:, :KV],
                    lhsT=hs_sb[dc][:, kc * 128 : (kc + 1) * 128],
                    rhs=wv_sb[dc][:, :],
                    start=(dc == 0),
                    stop=(dc == NDC - 1),
                )
            nc.vector.memset(
                va_sb[kc].rearrange("p (g w) -> p g w", g=G)[:, :, 64:65], 1.0
            )
            dst = va_sb[kc].rearrange("p (g w) -> p g w", g=G)[:, :, 0:64]
            srcv = vps[:, :KV].rearrange("p (g w) -> p g w", g=G)
            nc.vector.tensor_copy(dst, srcv)

        def exp_op(at_t, st, width, kc):
            if kc in DVE_KC:
                nc.vector.tensor_scalar_add(at_t[:, :width].bitcast(I16), st[:, :width], SCHR_B)
            else:
                nc.scalar.activation(
                    at_t[:, :width],
                    st[:, :width],
                    mybir.ActivationFunctionType.Exp,
                    scale=ACT_SCALE,
                )

        def norm(h, av, qb):
            """ot[h] rows = av[0:64] * (1/av[64]) broadcast.

            Evacuate the av PSUM bank in one fast fp16 cast so the bank frees
            immediately (av pool is only double-buffered); the rest of the
            chain runs from SBUF at 16-bit DVE rates. custom-DVE ops drop
            PSUM partition offsets, so the denominator comes from the SBUF
            copy as well."""
            avc = miscp.tile([65, 512], F16, tag="avc", name=f"avc{h}{qb}")
            nc.vector.tensor_copy(avc, av)
            dn = miscp.tile([1, 512], F32, tag="dn", name=f"dn{h}{qb}")
            nc.vector.tensor_copy(dn, avc[64:65, :])
            rd = miscp.tile([1, 512], F32, tag="rd", name=f"rd{h}{qb}")
            nc.vector.reciprocal_approx_fast(out=rd, in_=dn)
            bc = miscp.tile([64, 512], F32, tag="bc", name=f"bc{h}{qb}")
            nc.gpsimd.partition_broadcast(bc, rd)
            row = (h % 2) * 64
            nc.vector.tensor_mul(
                ot_sb[h // 2][row : row + 64, qb * 512 : (qb + 1) * 512],
                avc[0:64, :],
                bc,
            )

        # o_proj split into an early partial pass (filler work during the
        # last two pairs) and a short tail: partial = sum over cc subset into
        # PSUM, evacuated to SBUF so the PSUM slot frees; tail adds the
        # remaining cc terms.
        opp = ctx.enter_context(tc.tile_pool(name="opp", bufs=1))
        oproj_fsum = {}

        def oproj_partial(ec, sb2, ccs):
            m = min(128, H - ec * 128)
            ft = psp.tile([128, SQ], F32, tag="st", name=f"ftp{ec}{sb2}")[:, :512]
            for i, cc in enumerate(ccs):
                k = min(128, H - cc * 128)
                nc.tensor.matmul(
                    ft[:m, :],
                    lhsT=wo_sb[cc][:k, ec * 128 : ec * 128 + m],
                    rhs=ot_sb[cc][:k, sb2 * 512 : (sb2 + 1) * 512],
                    start=(i == 0),
                    stop=(i == len(ccs) - 1),
                )
            fs = opp.tile([128, 512], F32, tag=f"fs{ec}{sb2}", name=f"fs{ec}{sb2}")
            nc.vector.tensor_copy(fs[:m, :], ft[:m, :])
            oproj_fsum[(ec, sb2)] = (fs, ccs)

        # ---------------- preamble projections ----------------
        # minimal preamble so the exp stream (the bottleneck engine) starts
        # as early as possible; everything else streams in as fillers.
        # pair 0 only needs KV group 0, so the g2 K chunks are deferred.
        k_proj(0, chunks=(0,))
        q_proj(0)
        v_proj(0)
        v_proj(1)

        # ---------------- attention ----------------
        # filler projections interleaved into the attention loop, keyed by
        # (pair, qb, kc) -> list of thunks. They keep the PE dense while
        # ACT/DVE chew on the exp stream. Fillers with DVE-side work are
        # staggered away from the DVE-exp chunks (DVE queue is in-order).
        fillers = {}
        fillers.setdefault((0, 0, 0), []).append(lambda: k_proj(1, chunks=(0,)))
        fillers.setdefault((0, 0, 1), []).append(lambda: v_proj(2))
        fillers.setdefault((0, 0, 1), []).append(lambda: v_proj(3))
        for kc in range(4, NKC):
            fillers.setdefault((0, 0, kc - 2), []).append(lambda kc=kc: v_proj(kc))
        fillers.setdefault((0, 1, 3), []).append(lambda: q_proj(1))
        fillers.setdefault((1, 0, 3), []).append(lambda: q_proj(2))
        fillers.setdefault((1, 0, 6), []).append(lambda: k_proj(0, chunks=(1,)))
        fillers.setdefault((1, 1, 6), []).append(lambda: k_proj(1, chunks=(1,)))
        fillers.setdefault((2, 0, 3), []).append(lambda: q_proj(3))
        fillers.setdefault((3, 0, 3), []).append(lambda: q_proj(4))
        # o_proj partials over already-finished ot chunks (cc 0-2 exist once
        # pair 2 is done; cc 3 once pair 3 is done)
        for i, (ec, sb2) in enumerate([(0, 0), (0, 1), (1, 0), (1, 1), (2, 0)]):
            fillers.setdefault((3, i % 2, 5 + 4 * (i // 2)), []).append(
                lambda ec=ec, sb2=sb2: oproj_partial(ec, sb2, (0, 1, 2))
            )

        for pair in range(4):
            hA = 2 * pair
            hB = hA + 1
            gA = hA // 3
            gB = hB // 3
            c = pair
            for qb in range(2):
                avA = avp.tile([65, 512], F32, tag="av", name=f"avA{pair}{qb}")
                avB = avp.tile([65, 512], F32, tag="av", name=f"avB{pair}{qb}")
                pend = None
                for kc in range(NKC):
                    for f in fillers.get((pair, qb, kc), ()):
                        f()
                    kcs = slice(kc * 128, (kc + 1) * 128)
                    qbs = slice(qb * 512, (qb + 1) * 512)
                    st = psp.tile([128, 1024], F32, tag="st", name=f"st{pair}{qb}{kc}")
                    nc.tensor.matmul(
                        st[:, 0:512],
                        lhsT=ktd_sb[gA][0:64, kcs],
                        rhs=qt_sb[c][0:64, qbs],
                        start=True,
                        stop=True,
                    )
                    nc.tensor.matmul(
                        st[:, 512:1024],
                        lhsT=ktd_sb[gB][64:128, kcs],
                        rhs=qt_sb[c][64:128, qbs],
                        start=True,
                        stop=True,
                    )
                    at_t = attnp.tile([128, 1024], F16, tag="at", name=f"at{pair}{qb}{kc}")
                    exp_op(at_t, st, 1024, kc)
                    if pend is not None:
                        pat, pkc = pend
                        nc.tensor.matmul(
                            avA,
                            lhsT=va_sb[pkc][:, gA * 65 : gA * 65 + 65],
                            rhs=pat[:, 0:512],
                            start=(pkc == 0),
                            stop=False,
                        )
                        nc.tensor.matmul(
                            avB,
                            lhsT=va_sb[pkc][:, gB * 65 : gB * 65 + 65],
                            rhs=pat[:, 512:1024],
                            start=(pkc == 0),
                            stop=False,
                        )
                    pend = (at_t, kc)
                pat, pkc = pend
                nc.tensor.matmul(
                    avA,
                    lhsT=va_sb[pkc][:, gA * 65 : gA * 65 + 65],
                    rhs=pat[:, 0:512],
                    start=False,
                    stop=True,
                )
                nc.tensor.matmul(
                    avB,
                    lhsT=va_sb[pkc][:, gB * 65 : gB * 65 + 65],
                    rhs=pat[:, 512:1024],
                    start=False,
                    stop=True,
                )
                norm(hA, avA, qb)
                norm(hB, avB, qb)

        # pair 4: single head 8, qb0/qb1 processed together (row-packed via
        # the duplicated qt rows), so it runs at the same rate as full pairs.
        g2 = 2
        for i, (ec, sb2) in enumerate([(2, 1), (3, 0), (3, 1), (4, 0), (4, 1)]):
            fillers.setdefault((4, 0, 2 + 3 * i), []).append(
                lambda ec=ec, sb2=sb2: oproj_partial(ec, sb2, (0, 1, 2, 3))
            )
        av0 = avp.tile([65, 512], F32, tag="av", name="av8q0")
        av1 = avp.tile([65, 512], F32, tag="av", name="av8q1")
        pend = None
        for kc in range(NKC):
            for f in fillers.get((4, 0, kc), ()):
                f()
            kcs = slice(kc * 128, (kc + 1) * 128)
            st = psp.tile([128, 1024], F32, tag="st", name=f"st8{kc}")
            nc.tensor.matmul(
                st[:, 0:512],
                lhsT=ktd_sb[g2][0:64, kcs],
                rhs=qt_sb[4][0:64, 0:512],
                start=True,
                stop=True,
            )
            nc.tensor.matmul(
                st[:, 512:1024],
                lhsT=ktd_sb[g2][64:128, kcs],
                rhs=qt_sb[4][64:128, 512:1024],
                start=True,
                stop=True,
            )
            at_t = attnp.tile([128, 1024], F16, tag="at", name=f"at8{kc}")
            exp_op(at_t, st, 1024, kc)
            if pend is not None:
                pat, pkc = pend
                nc.tensor.matmul(
                    av0,
                    lhsT=va_sb[pkc][:, g2 * 65 : g2 * 65 + 65],
                    rhs=pat[:, 0:512],
                    start=(pkc == 0),
                    stop=False,
                )
                nc.tensor.matmul(
                    av1,
                    lhsT=va_sb[pkc][:, g2 * 65 : g2 * 65 + 65],
                    rhs=pat[:, 512:1024],
                    start=(pkc == 0),
                    stop=False,
                )
            pend = (at_t, kc)
        pat, pkc = pend
        nc.tensor.matmul(
            av0,
            lhsT=va_sb[pkc][:, g2 * 65 : g2 * 65 + 65],
            rhs=pat[:, 0:512],
            start=False,
            stop=True,
        )
        nc.tensor.matmul(
            av1,
            lhsT=va_sb[pkc][:, g2 * 65 : g2 * 65 + 65],
            rhs=pat[:, 512:1024],
            start=False,
            stop=True,
        )
        norm(8, av0, 0)
        norm(8, av1, 1)

        # ---------------- output projection tail ----------------
        # finish each (ec, sb2) block: accumulate the cc terms not covered by
        # the early partial pass, add the evacuated partial, store.
        for ec in range(NEC):
            m = min(128, H - ec * 128)
            for sb2 in range(2):
                fs, done_ccs = oproj_fsum.get((ec, sb2), (None, ()))
                rest = [cc for cc in range(NEC) if cc not in done_ccs]
                ft = psp.tile([128, SQ], F32, tag="st", name=f"ft{ec}{sb2}")[:, :512]
                for i, cc in enumerate(rest):
                    k = min(128, H - cc * 128)
                    nc.tensor.matmul(
                        ft[:m, :],
                        lhsT=wo_sb[cc][:k, ec * 128 : ec * 128 + m],
                        rhs=ot_sb[cc][:k, sb2 * 512 : (sb2 + 1) * 512],
                        start=(i == 0),
                        stop=(i == len(rest) - 1),
                    )
                fts = miscp.tile([128, 512], F32, tag="fts", name=f"fts{ec}{sb2}")
                if fs is not None:
                    nc.vector.tensor_add(fts[:m, :], ft[:m, :], fs[:m, :])
                else:
                    nc.scalar.copy(fts[:m, :], ft[:m, :])
                qeng[(ec * 2 + sb2) % 4].dma_start(
                    out=out[ec * 128 : ec * 128 + m, sb2 * 512 : (sb2 + 1) * 512],
                    in_=fts[:m, :],
                )


_NC_CACHE = {}


def _get_nc():
    if "nc" not in _NC_CACHE:
        _NC_CACHE["nc"] = _build_bass()
    return _NC_CACHE["nc"]


def kernel(hidden_states, wq, wk, wv, wo):
    cos4, sin2 = _rope_tables()  # fp32 [128, S]

    wq16 = np.zeros((HP, H), np.float16)
    wq16[:H] = wq.T.astype(np.float16)
    wk16 = np.zeros((HP, KV), np.float16)
    wk16[:H] = wk.T.astype(np.float16)
    wv16 = np.zeros((HP, KV), np.float16)
    wv16[:H] = wv.T.astype(np.float16)
    wo16 = wo.T.astype(np.float16)

    cosk0 = cos4.astype(np.float16)
    sink0 = sin2.astype(np.float16)
    cosk1 = np.roll(cosk0, -SQ, axis=1)
    sink1 = np.roll(sink0, -SQ, axis=1)
    # Q-side tables carry the Schraudolph scale so scores land pre-multiplied;
    # per seq-half the q columns are original positions [half*SQ, (half+1)*SQ)
    cosq0 = (cos4[:, :SQ] * SCHR_A).astype(np.float16)
    sinq0 = (sin2[:, :SQ] * SCHR_A).astype(np.float16)
    cosq1 = (cos4[:, SQ:] * SCHR_A).astype(np.float16)
    sinq1 = (sin2[:, SQ:] * SCHR_A).astype(np.float16)

    in_maps = []
    core_ids = list(range(8))
    for c in core_ids:
        b, half = c // 2, c % 2
        hsT16 = np.zeros((HP, S), np.float16)
        hsT16[:H] = hidden_states[b].T.astype(np.float16)
        if half == 1:
            # roll so this core's queries sit at columns [0, SQ); keys keep
            # their correct rope position via the equally-rolled cos/sin.
            hsT16 = np.roll(hsT16, -SQ, axis=1)
        in_maps.append(
            {
                "hsT": hsT16,
                "wqT": wq16,
                "wkT": wk16,
                "wvT": wv16,
                "woT": wo16,
                "cosq": cosq0 if half == 0 else cosq1,
                "sinq": sinq0 if half == 0 else sinq1,
                "cosk": cosk0 if half == 0 else cosk1,
                "sink": sink0 if half == 0 else sink1,
            }
        )

    global _LAST_IN_MAPS
    _LAST_IN_MAPS = in_maps
    nc = _get_nc()
    res = run_bass_kernel_spmd(nc, in_maps, core_ids=core_ids)

    out = np.empty((B, S, H), np.float32)
    for c in core_ids:
        b, half = c // 2, c % 2
        out[b, half * SQ : (half + 1) * SQ, :] = res.results[c]["o"].T
    return out


if __name__ == "__main__":
    rng = np.random.default_rng(0)
    hs = rng.standard_normal((B, S, H), dtype=np.float32)
    s = 1.0 / np.sqrt(H)
    wq = rng.standard_normal((H, H), dtype=np.float32) * s
    wk = rng.standard_normal((KV, H), dtype=np.float32) * s
    wv = rng.standard_normal((KV, H), dtype=np.float32) * s
    wo = rng.standard_normal((H, H), dtype=np.float32) * s
    o = kernel(hidden_states=hs, wq=wq, wk=wk, wv=wv, wo=wo)
    print(o.shape, o.dtype, np.abs(o).mean())
